# revision 23
# baseline (speedup 1.0000x reference)
"""Trainium2 Bass kernel for a CrossAttentionBlock.

Per-core computation (data-parallel over batch, B=8 -> 8 NeuronCores):
  qc   = conv2d_3x3_same(q, conv_w)                  [64, 48, 48]
  qs   = qc  as [C=64, S=2304]  (chan-major layout)
  qp   = (rmsnorm(qs, nq_w) @ wq.T + bq) / 4         per-pixel RMS over C
  kp   = rmsnorm(k, nk_w) @ wk.T + bk
  per head h (4 heads, d=16):  S_h = qp_h^T kp_h     [2304, 2304]
  out  = 0.25 * sum_h softmax_j(S_h) @ v^T           [2304, 64]

Implementation notes:
  - all big matmuls run as float32r (TF32-ish, 1 cycle/row on the PE)
  - heads are "spread": head h lives on SBUF partitions 32h..32h+15 so the
    K=16 score matmuls use PE row-groups via tile_position and overlap
  - softmax denominator comes for free from a ones-column appended to v^T
  - exp runs on the scalar engine in [128, 1024] tiles (the bottleneck)
  - 1/Z on the vector engine (reciprocal); rsqrt for RMS via exp(-0.5 ln x)
    so the whole kernel uses a single ACT table set
"""

import sys
import types

for _p in ("/opt/trn_rl_repo", "/root/.axon_site"):
    if _p not in sys.path:
        sys.path.insert(0, _p)

import numpy as np


def _ensure_ntff_hook():
    """Register the axon NTFF profile hook if the image's antenv lacks it."""
    try:
        import antenv.axon_hooks  # noqa: F401

        return
    except ImportError:
        pass
    try:
        from trn_agent_boot.trn_boot import _ntff_profile_via_ctypes

        hook = _ntff_profile_via_ctypes("/opt/axon/libaxon_pjrt.so")
        mod = types.ModuleType("antenv.axon_hooks")
        mod.get_axon_ntff_profile_hook = lambda: hook
        mod.set_axon_ntff_profile_hook = lambda h: None
        sys.modules["antenv.axon_hooks"] = mod
    except Exception:
        pass


_ensure_ntff_hook()

import concourse.bacc as bacc
import concourse.tile as tile
from concourse import mybir
from concourse.bass_utils import run_bass_kernel_spmd
from concourse.masks import make_identity

F32 = mybir.dt.float32
F32R = mybir.dt.float32r
BF16 = mybir.dt.bfloat16
AF = mybir.ActivationFunctionType
OP = mybir.AluOpType

N_CORES = 8
C = 64
H = W = 48
S = H * W  # 2304
NH = 4
HD = 16
SCALE = 1.0 / np.sqrt(HD)  # 0.25
EPS = 1.1920929e-07
WP = W + 2  # padded width 50
NJT = S // 128  # 18 j-tiles
# i-chunks: 4 x 512 + 1 x 256
ICHUNKS = [(0, 512), (512, 512), (1024, 512), (1536, 512), (2048, 256)]


def r32(ap):
    return ap.bitcast(F32R)


def build(stage=None):
    import os

    stage = stage or os.environ.get("K_STAGE", "full")
    nc = bacc.Bacc(
        "TRN2", target_bir_lowering=False, debug=False, num_devices=N_CORES
    )

    q_d = nc.dram_tensor("q", [C, S], F32, kind="ExternalInput").ap()
    k_d = nc.dram_tensor("k", [C, S], F32, kind="ExternalInput").ap()
    v_d = nc.dram_tensor("v", [C, S], F32, kind="ExternalInput").ap()
    cw_d = nc.dram_tensor("conv_w", [C, C * 9], F32, kind="ExternalInput").ap()
    wq_d = nc.dram_tensor("wq", [C, C], F32, kind="ExternalInput").ap()
    wk_d = nc.dram_tensor("wk", [C, C], F32, kind="ExternalInput").ap()
    bq_d = nc.dram_tensor("bq", [C, 1], F32, kind="ExternalInput").ap()
    bk_d = nc.dram_tensor("bk", [C, 1], F32, kind="ExternalInput").ap()
    nq_d = nc.dram_tensor("nq_w", [C, 1], F32, kind="ExternalInput").ap()
    nk_d = nc.dram_tensor("nk_w", [C, 1], F32, kind="ExternalInput").ap()
    out_d = nc.dram_tensor("out", [NJT, 128, C], F32, kind="ExternalOutput").ap()

    with tile.TileContext(nc) as tc:
        with (
            tc.tile_pool(name="const", bufs=1) as const,
            tc.tile_pool(name="work", bufs=3) as work,
            tc.tile_pool(name="small", bufs=4) as small,
        ):
            # ---------------- input DMAs (spread over queues) ----------------
            q_in = const.tile([C, S], F32)
            k_in = const.tile([C, S], F32)
            v_in = const.tile([C, S], F32)
            cw_sb = const.tile([C, C * 9], F32)
            wq_sb = const.tile([C, C], F32)
            wk_sb = const.tile([C, C], F32)

            nq_col = const.tile([C, 1], F32)
            nk_col = const.tile([C, 1], F32)
            # q/k split across queues: a single-queue 590KB DMA lands at
            # ~15us, which was the binding preamble constraint
            HS = S // 2
            nc.scalar.dma_start(out=k_in[:, 0:HS], in_=k_d[:, 0:HS])
            nc.scalar.dma_start(out=q_in[:, 0:HS], in_=q_d[:, 0:HS])
            nc.sync.dma_start(out=wq_sb, in_=wq_d)
            nc.sync.dma_start(out=wk_sb, in_=wk_d)
            nc.sync.dma_start(out=nq_col, in_=nq_d)
            nc.sync.dma_start(out=nk_col, in_=nk_d)
            nc.sync.dma_start(out=k_in[:, HS:S], in_=k_d[:, HS:S])
            nc.sync.dma_start(out=q_in[:, HS:S], in_=q_d[:, HS:S])
            nc.gpsimd.dma_start(out=cw_sb, in_=cw_d)
            nc.gpsimd.dma_start(out=v_in, in_=v_d)

            # ---------------- constants ----------------
            ident = const.tile([128, 128], F32)
            make_identity(nc, ident)
            ones_sb = const.tile([C, 128], BF16)
            nc.gpsimd.memset(ones_sb, 1.0)
            eps_col = const.tile([128, 1], F32)
            nc.gpsimd.memset(eps_col, EPS)
            zero_col = const.tile([128, 1], F32)
            nc.gpsimd.memset(zero_col, 0.0)

            # persistent big tiles (bf16 for matmul streaming operands)
            # zero-padded conv input (+2 slack so shifted row-slices stay in range)
            qpad = const.tile([C, WP * (H + 2) + 2], BF16)
            qc_sb = const.tile([C, S], BF16)  # conv output
            qsq_sb = const.tile([C, S], BF16)  # conv output squared
            ksq_sb = const.tile([C, S], BF16)
            k_r = const.tile([C, S], BF16)  # bf16 copy of k for the matmul
            qp_pack = const.tile([128, S], BF16)  # head h at partitions 32h..+15
            kp_pack = const.tile([128, S], BF16)
            # per-head zero-padded qp slabs: full-K=128 score matmuls (keeps
            # the PE activity monitor warm; tile_position matmuls do not --
            # measured: row-group scores leave HAM at K=4/8 for the whole
            # main loop, 342us vs 294us)
            qp_all = const.tile([128, NH * S], BF16)
            # proj/r in bf16: they only feed the bf16 packs, and 16-bit
            # doubles DVE throughput for the copies and pack muls
            proj_q = const.tile([128, S], BF16)
            proj_k = const.tile([128, S], BF16)
            rln_q = const.tile([128, S], F32)
            rln_k = const.tile([128, S], F32)
            r_q = const.tile([128, S], BF16)
            r_k = const.tile([128, S], BF16)
            vT1 = const.tile([128, NJT * (C + 1)], BF16)  # [v^T | 1] per j-tile
            acc = const.tile([128, NJT * C], F32)  # final output accumulator
            wq_sp = const.tile([C, 128], BF16)  # spread projection weights
            wk_sp = const.tile([C, 128], BF16)
            bq_sp = const.tile([128, 1], F32)
            bk_sp = const.tile([128, 1], F32)

            # warm_sb memset FIRST and on gpsimd: the DVE FIFO has ~6us of
            # framework preamble that would delay the PE warm-up
            warm_sb = const.tile([128, 512], BF16)
            nc.gpsimd.memset(warm_sb, 0.125)
            nc.vector.memset(qpad, 0.0)
            nc.vector.memset(vT1, 1.0)
            nc.gpsimd.memset(qp_all, 0.0)
            nc.gpsimd.memset(wq_sp, 0.0)
            nc.gpsimd.memset(wk_sp, 0.0)
            nc.gpsimd.memset(bq_sp, 0.0)
            nc.gpsimd.memset(bk_sp, 0.0)

            with tc.tile_pool(name="pre_ps", bufs=2, space="PSUM") as pre_ps:
                # ---- PE warm-up: dense full-array matmuls so the clock
                # gate (HAM) reaches 2.4 GHz before the real work arrives
                warm_ps = pre_ps.tile([128, 512], F32, tag="cps")
                for _ in range(16):
                    nc.tensor.matmul(
                        warm_ps,
                        lhsT=warm_sb[:, 0:128],
                        rhs=warm_sb,
                        start=True,
                        stop=True,
                    )

                # ---- transpose + fold norm weights into projections ----
                wqT_sb = const.tile([C, C], F32)
                wkT_sb = const.tile([C, C], F32)
                for w_sb, wT_sb, n_col in (
                    (wq_sb, wqT_sb, nq_col),
                    (wk_sb, wkT_sb, nk_col),
                ):
                    tp = pre_ps.tile([128, 65], F32, tag="tps")
                    nc.tensor.transpose(tp[0:C, 0:C], w_sb, ident[0:C, 0:C])
                    nc.vector.tensor_scalar(
                        out=wT_sb,
                        in0=tp[0:C, 0:C],
                        scalar1=n_col,
                        scalar2=None,
                        op0=OP.mult,
                    )
                # spread head h columns to 32h..32h+15
                for h in range(NH):
                    nc.vector.tensor_copy(
                        out=wq_sp[:, 32 * h : 32 * h + HD],
                        in_=wqT_sb[:, HD * h : HD * (h + 1)],
                    )
                    nc.vector.tensor_copy(
                        out=wk_sp[:, 32 * h : 32 * h + HD],
                        in_=wkT_sb[:, HD * h : HD * (h + 1)],
                    )
                    # engine partition access must be 32-aligned; DMA instead
                    nc.gpsimd.dma_start(
                        out=bq_sp[32 * h : 32 * h + HD, :],
                        in_=bq_d[HD * h : HD * (h + 1), :],
                    )
                    nc.gpsimd.dma_start(
                        out=bk_sp[32 * h : 32 * h + HD, :],
                        in_=bk_d[HD * h : HD * (h + 1), :],
                    )
                # fold the 1/sqrt(head_dim) factor into the q bias
                nc.vector.tensor_scalar(
                    out=bq_sp, in0=bq_sp, scalar1=SCALE, scalar2=None, op0=OP.mult
                )

                # ---- k squared + bf16 k for the projection (early: the
                # k path feeds the scalar Ln chain long before conv ends)
                nc.vector.tensor_mul(ksq_sb, k_in, k_in)
                nc.scalar.copy(out=k_r, in_=k_in)

                def phase1(src_sb, sq_sb, w_sp, proj_f, rln_f, c0, cw_):
                    sl = slice(c0, c0 + cw_)
                    pp = pre_ps.tile([128, 512], F32, tag="pps")
                    sp = pre_ps.tile([128, 512], F32, tag="sps")
                    nc.tensor.matmul(
                        pp[:, 0:cw_],
                        lhsT=w_sp,
                        rhs=src_sb[:, sl],
                        start=True,
                        stop=True,
                    )
                    nc.tensor.matmul(
                        sp[:, 0:cw_],
                        lhsT=ones_sb,
                        rhs=sq_sb[:, sl],
                        start=True,
                        stop=True,
                    )
                    nc.scalar.activation(
                        out=rln_f[:, sl],
                        in_=sp[:, 0:cw_],
                        func=AF.Ln,
                        scale=1.0 / C,
                        bias=eps_col,
                    )
                    nc.vector.tensor_copy(out=proj_f[:, sl], in_=pp[:, 0:cw_])

                # k-path phase 1 first: PE cost is tiny and it unblocks the
                # scalar Ln chain at ~10us instead of after the whole conv
                for c0, cw_ in ICHUNKS:
                    phase1(k_r, ksq_sb, wk_sp, proj_k, rln_k, c0, cw_)

                # ---- copy q into padded plane (rows shifted by 1,1) ----
                dst = qpad[:, WP + 1 : WP + 1 + H * WP].rearrange(
                    "p (r w) -> p r w", w=WP
                )[:, :, 0:W]
                nc.vector.tensor_copy(
                    out=dst, in_=q_in.rearrange("p (r w) -> p r w", w=W)
                )

                # ---- transpose conv weights: per tap t, [o,i] -> [i,o] ----
                cwT_sb = const.tile([C, 9 * C], BF16)
                for t in range(9):
                    tp = pre_ps.tile([128, 65], F32, tag="tps")
                    nc.tensor.transpose(
                        tp[0:C, 0:C],
                        cw_sb.rearrange("p (i t) -> p t i", t=9)[:, t, :],
                        ident[0:C, 0:C],
                    )
                    nc.vector.tensor_copy(
                        out=cwT_sb[:, t * C : (t + 1) * C], in_=tp[0:C, 0:C]
                    )

                # ---- conv as 9 accumulated shifted matmuls, interleaved
                # with q-path phase 1 as soon as each i-chunk is covered ----
                def conv_chunk(r0, nr):
                    cp = pre_ps.tile([C, 512], F32, tag="cps")
                    n_out = nr * W
                    for t in range(9):
                        ky, kx = divmod(t, 3)
                        src = qpad[
                            :,
                            (r0 + ky) * WP + kx : (r0 + ky) * WP + kx + nr * WP,
                        ].rearrange("p (r w) -> p r w", w=WP)[:, :, 0:W]
                        nc.tensor.matmul(
                            cp[:, 0:n_out],
                            lhsT=cwT_sb[:, t * C : (t + 1) * C],
                            rhs=src,
                            start=(t == 0),
                            stop=(t == 8),
                        )
                    sl = slice(r0 * W, r0 * W + n_out)
                    nc.vector.tensor_copy(out=qc_sb[:, sl], in_=cp[:, 0:n_out])
                    nc.vector.tensor_mul(qsq_sb[:, sl], qc_sb[:, sl], qc_sb[:, sl])

                row_chunks = [(0, 10), (10, 10), (20, 10), (30, 10), (40, 8)]
                conv_chunk(*row_chunks[0])
                conv_chunk(*row_chunks[1])
                for ci in range(5):
                    # conv chunks 0..ci+1 cover i < 480*(ci+2), which always
                    # contains the q i-chunk ci (ends at 512*(ci+1) <= 2304)
                    if ci + 2 < len(row_chunks):
                        conv_chunk(*row_chunks[ci + 2])
                    c0, cw_ = ICHUNKS[ci]
                    phase1(qc_sb, qsq_sb, wq_sp, proj_q, rln_q, c0, cw_)

                # ---- v^T blocks (ones column at index 64 of each group);
                # placed here so they fill the PE idle while the scalar
                # engine runs the r-factor exps ----
                for jt in range(NJT):
                    tp = pre_ps.tile([128, 65], F32, tag="tps")
                    nc.tensor.transpose(
                        tp[:, 0:C],
                        v_in[:, jt * 128 : (jt + 1) * 128],
                        ident[0:C, 0:C],
                    )
                    nc.vector.tensor_copy(
                        out=vT1[:, jt * 65 : jt * 65 + C], in_=tp[:, 0:C]
                    )
                # exp ops bias on this dep tile so every preamble Exp is
                # scheduled after every Ln (keeps one ACT table set loaded)
                zero_dep = const.tile([128, 1], F32)
                nc.vector.tensor_mul(
                    zero_dep, rln_q[:, S - 1 : S], rln_k[:, S - 1 : S]
                )
                nc.vector.tensor_scalar(
                    out=zero_dep, in0=zero_dep, scalar1=0.0, scalar2=None,
                    op0=OP.mult,
                )
                # phases 2+3 per chunk: r = exp(-0.5 ln(mean+eps)); scale,
                # bias, pack; k first so the main loop can start on qp chunk 0
                for flow, (proj_f, rln_f, r_f, b_sp, dst_pack, post_mul) in (
                    ("k", (proj_k, rln_k, r_k, bk_sp, kp_pack, None)),
                    ("q", (proj_q, rln_q, r_q, bq_sp, qp_pack, SCALE)),
                ):
                    # one big Exp per flow: amortizes the per-inst overhead
                    nc.scalar.activation(
                        out=r_f,
                        in_=rln_f,
                        func=AF.Exp,
                        scale=-0.5,
                        bias=zero_dep,
                    )
                    for c0, cw_ in ICHUNKS:
                        sl = slice(c0, c0 + cw_)
                        nc.vector.tensor_mul(
                            dst_pack[:, sl], proj_f[:, sl], r_f[:, sl]
                        )
                        if post_mul is not None:
                            nc.vector.tensor_scalar(
                                out=dst_pack[:, sl],
                                in0=dst_pack[:, sl],
                                scalar1=post_mul,
                                scalar2=b_sp,
                                op0=OP.mult,
                                op1=OP.add,
                            )
                        else:
                            nc.vector.tensor_scalar(
                                out=dst_pack[:, sl],
                                in0=dst_pack[:, sl],
                                scalar1=b_sp,
                                scalar2=None,
                                op0=OP.add,
                            )
                        if flow == "q":
                            # spread heads into the zero-padded slabs via
                            # SBUF->SBUF DMA on idle queues (16-lane DVE
                            # copies were ~10us of serialized vector time)
                            for h in range(NH):
                                eng = (nc.sync, nc.gpsimd)[h % 2]
                                eng.dma_start(
                                    out=qp_all[
                                        32 * h : 32 * h + HD,
                                        h * S + c0 : h * S + c0 + cw_,
                                    ],
                                    in_=qp_pack[32 * h : 32 * h + HD, sl],
                                )
            # ---------------- main attention loop ----------------
            if stage == "pre":
                nc.sync.dma_start(
                    out=out_d.rearrange("t p d -> p t d"),
                    in_=qp_pack[:, 0 : NJT * C].rearrange("p (t d) -> p t d", d=C),
                )
            if stage == "mini":
                main_iters = 2
            elif stage.startswith("it"):
                main_iters = int(stage[2:])
            else:
                main_iters = 2 * len(ICHUNKS)
            with (
                tc.tile_pool(name="psS", bufs=2, space="PSUM") as psS_pool,
                tc.tile_pool(name="psO", bufs=1, space="PSUM") as psO_pool,
                tc.tile_pool(name="psT", bufs=2, space="PSUM") as psT_pool,
            ):
                for hp in range(2):  # head pairs
                    if stage == "pre":
                        break
                    for ic, (i0, iw) in enumerate(ICHUNKS):
                        if hp * len(ICHUNKS) + ic >= main_iters:
                            break
                        psO = psO_pool.tile([65, 1024], F32, tag="O")
                        for jt in range(NJT):
                            psS = psS_pool.tile([128, 1024], F32, tag="S")
                            for hh in range(2):
                                h = 2 * hp + hh
                                # full-K matmul: kp_pack is zero off-head, the
                                # qp slab is zero off-head, so cross terms die
                                # psum matmul outputs must be bank-aligned
                                nc.tensor.matmul(
                                    psS[:, 512 * hh : 512 * hh + iw],
                                    lhsT=kp_pack[:, jt * 128 : (jt + 1) * 128],
                                    rhs=qp_all[:, h * S + i0 : h * S + i0 + iw],
                                    start=True,
                                    stop=True,
                                )
                            expT = work.tile([128, 1024], BF16, tag="expT")
                            if iw == 512:
                                nc.scalar.activation(
                                    out=expT[:, 0:1024],
                                    in_=psS[:, 0:1024],
                                    func=AF.Exp,
                                    bias=zero_col,
                                )
                            else:
                                # one activation over both head slices via a
                                # strided AP: halves the per-inst overhead
                                nc.scalar.activation(
                                    out=expT.rearrange(
                                        "p (b c) -> p b c", b=2
                                    )[:, :, 0:iw],
                                    in_=psS.rearrange(
                                        "p (b c) -> p b c", b=2
                                    )[:, :, 0:iw],
                                    func=AF.Exp,
                                    bias=zero_col,
                                )
                            for hh in range(2):
                                nc.tensor.matmul(
                                    psO[:, 512 * hh : 512 * hh + iw],
                                    lhsT=vT1[:, jt * 65 : (jt + 1) * 65],
                                    rhs=expT[:, 512 * hh : 512 * hh + iw],
                                    start=(jt == 0),
                                    stop=(jt == NJT - 1),
                                )
                        # ---- normalize and accumulate this chunk ----
                        for hh in range(2):
                            h = 2 * hp + hh
                            o_sb = work.tile([65, 512], F32, tag="Osb")
                            nc.vector.tensor_copy(
                                out=o_sb[:, 0:iw],
                                in_=psO[:, 512 * hh : 512 * hh + iw],
                            )
                            for it in range(iw // 128):
                                g = ic * 4 + it
                                tp = psT_pool.tile([128, 65], F32, tag="T")
                                nc.tensor.transpose(
                                    tp,
                                    o_sb[:, it * 128 : (it + 1) * 128],
                                    ident[0:65, 0:65],
                                )
                                rz = small.tile([128, 1], F32, tag="rz")
                                nc.vector.reciprocal(rz, tp[:, C : C + 1])
                                a_sl = acc[:, g * C : (g + 1) * C]
                                if h == 0:
                                    nc.vector.tensor_scalar(
                                        out=a_sl,
                                        in0=tp[:, 0:C],
                                        scalar1=rz,
                                        scalar2=0.25,
                                        op0=OP.mult,
                                        op1=OP.mult,
                                    )
                                else:
                                    tmp = small.tile([128, C], F32, tag="tmp")
                                    nc.vector.tensor_scalar(
                                        out=tmp,
                                        in0=tp[:, 0:C],
                                        scalar1=rz,
                                        scalar2=0.25,
                                        op0=OP.mult,
                                        op1=OP.mult,
                                    )
                                    nc.vector.tensor_add(a_sl, a_sl, tmp)
                        # chunk finalized on the second head pair: stream it
                        # out now so the final DMA is off the critical path
                        if hp == 1 and main_iters == 2 * len(ICHUNKS):
                            g0, g1 = i0 // 128, (i0 + iw) // 128
                            nc.sync.dma_start(
                                out=out_d[g0:g1].rearrange("t p d -> p t d"),
                                in_=acc[:, g0 * C : g1 * C].rearrange(
                                    "p (t d) -> p t d", d=C
                                ),
                            )

            # ---------------- write out ----------------
            if stage != "pre" and main_iters != 2 * len(ICHUNKS):
                nc.sync.dma_start(
                    out=out_d.rearrange("t p d -> p t d"),
                    in_=acc.rearrange("p (t d) -> p t d", d=C),
                )

    nc.compile()
    return nc


_NC_CACHE = None


def _get_nc():
    global _NC_CACHE
    if _NC_CACHE is None:
        _NC_CACHE = build()
    return _NC_CACHE


def make_in_maps(q, k, v, conv_w, nq_w, nk_w, wq, bq, wk, bk):
    B = q.shape[0]
    f = lambda a, s: np.ascontiguousarray(a, dtype=np.float32).reshape(s)
    shared = {
        "conv_w": f(conv_w, (C, C * 9)),
        "wq": f(wq, (C, C)),
        "wk": f(wk, (C, C)),
        "bq": f(bq, (C, 1)),
        "bk": f(bk, (C, 1)),
        "nq_w": f(nq_w, (C, 1)),
        "nk_w": f(nk_w, (C, 1)),
    }
    return [
        {
            "q": f(q[b], (C, S)),
            "k": f(k[b], (C, S)),
            "v": f(v[b], (C, S)),
            **shared,
        }
        for b in range(B)
    ]


def run(in_maps, **kwargs):
    nc = _get_nc()
    return run_bass_kernel_spmd(nc, in_maps, core_ids=list(range(N_CORES)), **kwargs)


def kernel(q, k, v, conv_w, nq_w, nk_w, wq, bq, wk, bk):
    res = run(make_in_maps(q, k, v, conv_w, nq_w, nk_w, wq, bq, wk, bk))
    return np.stack(
        [res.results[b]["out"].reshape(S, C) for b in range(q.shape[0])]
    )

